# revision 20
# baseline (speedup 1.0000x reference)
"""Trainium2 Bass kernel for nn_FPModule (retrieval_knn feature propagation).

Pipeline per core (2 point clouds each, data-parallel over 8 cores):
  1. e = -squared-distance matrix via PE matmul with 3-way bf16-split
     augmented coordinates (exact products, fp32 psum accumulation).
  2. Top-3 nearest coarse points per fine point via DVE InstMax (top-8 sorted).
  3. Normalized inverse-distance weights built as a sparse-row matrix W (fp16)
     with a fused compare+multiply; W transposed on the DMA xbar.
  4. interp^T = F^T-chunks @ W^T on PE (fp16), concat with x_skip^T (host-prep),
     then the 2-layer MLP entirely in transposed activation layout.
Output assembled/transposed on host. All dtype-critical selection math is fp32.
"""
import sys, os
for _p in ("/opt/trn_rl_repo", "/root/.axon_site/_ro/trn_rl_repo"):
    if os.path.isdir(_p) and _p not in sys.path:
        sys.path.insert(0, _p)

import numpy as np
import ml_dtypes

B, NC, NF, C = 16, 1024, 4096, 256
NCORES = 8
BPC = B // NCORES          # clouds per core
KAUG = 36                  # augmented contraction rows
BLK = 512                  # fine-block size
EPS = np.float32(1e-12)

bfl = ml_dtypes.bfloat16


def _split3(v64):
    """3-way bf16 split of fp64 values (captures ~24 mantissa bits)."""
    a = v64.astype(bfl)
    r = v64 - a.astype(np.float64)
    b = r.astype(bfl)
    r2 = r - b.astype(np.float64)
    c = r2.astype(bfl)
    return a, b, c


def _build_aug(pf32, pc32):
    """aug_fT [KAUG, NFl] bf16, aug_cT [KAUG, NCl] bf16 for e = -|pf-pc|^2.

    Per coordinate d, 12 k-rows in psum-accumulation order:
      6 cross pairs of splits of (2*pf_d) x splits of (pc_d):
         (fa,ca),(fa,cb),(fb,ca),(fa,cc),(fc,ca),(fb,cb)
      3 rows: splits of (-pf_d^2) x 1
      3 rows: 1 x splits of (-pc_d^2)
    """
    NFl, NCl = pf32.shape[0], pc32.shape[0]
    af = np.zeros((KAUG, NFl), dtype=bfl)
    ac = np.zeros((KAUG, NCl), dtype=bfl)
    onesf = np.ones(NFl, dtype=bfl)
    onesc = np.ones(NCl, dtype=bfl)
    pf = pf32.astype(np.float64)
    pc = pc32.astype(np.float64)
    k = 0
    for d in range(3):
        fa, fb, fc = _split3(2.0 * pf[:, d])
        ca, cb, cc_ = _split3(pc[:, d])
        for (u, v) in ((fa, ca), (fa, cb), (fb, ca), (fa, cc_), (fc, ca), (fb, cb)):
            af[k] = u; ac[k] = v; k += 1
        s1, s2, s3 = _split3(-(pf[:, d] ** 2))
        for s in (s1, s2, s3):
            af[k] = s; ac[k] = onesc; k += 1
        t1, t2, t3 = _split3(-(pc[:, d] ** 2))
        for t in (t1, t2, t3):
            af[k] = onesf; ac[k] = t; k += 1
    assert k == KAUG
    return af, ac


def build_core_kernel(nc, tc, cfg):
    """Emit one core's kernel (BPC clouds) inside an open TileContext."""
    import concourse.bass as bass
    from concourse import mybir

    F32 = mybir.dt.float32
    F16 = mybir.dt.float16
    BF16 = mybir.dt.bfloat16
    AF = mybir.ActivationFunctionType
    OP = mybir.AluOpType
    AX = mybir.AxisListType

    ncl, nfl, bpc = cfg["NC"], cfg["NF"], cfg["BPC"]
    nblk = nfl // BLK
    ncc = ncl // 128          # coarse chunks
    dbg = cfg.get("debug_outs", False)

    def act_recip(out, in_, scale, bias=0.0):
        """Raw ACT Reciprocal out = 1/(in*scale + bias); table accuracy ~1e-5
        (measured), far below the fp16 quantization of the consumers."""
        eng = nc.scalar
        inputs = [eng.lower_ap(in_)]
        for arg in (bias, scale, 0.0):   # bias, scale, alpha
            if isinstance(arg, bass.AP):
                inputs.append(eng.lower_ap(arg))
            else:
                inputs.append(mybir.ImmediateValue(dtype=mybir.dt.float32, value=arg))
        return eng.add_instruction(
            mybir.InstActivation(
                name=eng.bass.get_next_instruction_name(),
                func=AF.Reciprocal,
                ins=inputs,
                outs=[eng.lower_ap(out)],
            )
        )

    AUGF = nc.dram_tensor("AUGF", [bpc, KAUG, nfl], BF16, kind="ExternalInput").ap()
    AUGC = nc.dram_tensor("AUGC", [bpc, KAUG, ncl], BF16, kind="ExternalInput").ap()
    FT = nc.dram_tensor("FT", [bpc, ncl, C], F16, kind="ExternalInput").ap()
    XSKT = nc.dram_tensor("XSKT", [bpc, C, nfl], F16, kind="ExternalInput").ap()
    W1D = nc.dram_tensor("W1D", [512, 512], F16, kind="ExternalInput").ap()
    W2D = nc.dram_tensor("W2D", [512, 256], F16, kind="ExternalInput").ap()
    B1D = nc.dram_tensor("B1D", [128, 4], F32, kind="ExternalInput").ap()
    B2D = nc.dram_tensor("B2D", [128, 2], F32, kind="ExternalInput").ap()
    OUTD = nc.dram_tensor("OUTD", [bpc, C, nfl], F32, kind="ExternalOutput").ap()
    if dbg:
        EDBG = nc.dram_tensor("EDBG", [bpc, nfl, ncl], F32, kind="ExternalOutput").ap()
        M8DBG = nc.dram_tensor("M8DBG", [bpc, nblk, 128, 32], F32, kind="ExternalOutput").ap()
        WTDBG = nc.dram_tensor("WTDBG", [bpc, nblk, 128, ncc, BLK], F16, kind="ExternalOutput").ap()

    use_div = cfg.get("use_div", False)

    import contextlib
    ctx = contextlib.ExitStack()
    cst = ctx.enter_context(tc.tile_pool(name="cst", bufs=1))
    pcl = ctx.enter_context(tc.tile_pool(name="pcl", bufs=2))
    pe_sb = ctx.enter_context(tc.tile_pool(name="pe_sb", bufs=8))
    pw16 = ctx.enter_context(tc.tile_pool(name="pw16", bufs=4))
    pwt = ctx.enter_context(tc.tile_pool(name="pwt", bufs=4))
    psml = ctx.enter_context(tc.tile_pool(name="psml", bufs=10))
    prsc = ctx.enter_context(tc.tile_pool(name="prsc", bufs=4))
    pht = ctx.enter_context(tc.tile_pool(name="pht", bufs=3))
    ph1 = ctx.enter_context(tc.tile_pool(name="ph1", bufs=2))
    ph2 = ctx.enter_context(tc.tile_pool(name="ph2", bufs=2))
    psE = ctx.enter_context(tc.tile_pool(name="psE", bufs=2, space="PSUM"))
    psI = ctx.enter_context(tc.tile_pool(name="psI", bufs=1, space="PSUM"))
    psMM = ctx.enter_context(tc.tile_pool(name="psMM", bufs=2, space="PSUM"))

    # constants
    w1sb = cst.tile([128, 4, 512], F16, tag="w1sb")
    for kc in range(4):
        nc.sync.dma_start(w1sb[:, kc, :], W1D[kc * 128:(kc + 1) * 128, :])
    w2sb = cst.tile([128, 4, 256], F16, tag="w2sb")
    for kc in range(4):
        nc.sync.dma_start(w2sb[:, kc, :], W2D[kc * 128:(kc + 1) * 128, :])
    b1sb = cst.tile([128, 4], F32, tag="b1sb")
    nc.sync.dma_start(b1sb[:], B1D[:])
    b2sb = cst.tile([128, 2], F32, tag="b2sb")
    nc.sync.dma_start(b2sb[:], B2D[:])

    def load_cloud(cl):
        augf = pcl.tile([KAUG, nfl], BF16, tag="augf")
        nc.sync.dma_start(augf[:], AUGF[cl])
        augc = pcl.tile([KAUG, ncl], BF16, tag="augc")
        nc.sync.dma_start(augc[:], AUGC[cl])
        fsb = pcl.tile([128, ncc, C], F16, tag="fsb")
        for cc in range(ncc):
            nc.sync.dma_start(fsb[:, cc, :], FT[cl, cc * 128:(cc + 1) * 128, :])
        return augf, augc, fsb

    def emit_chain(cl, blk, res):
        augf, augc, fsb = res
        wt = pwt.tile([128, ncc, BLK], F16, tag="wt")
        for ft in range(4):
            f0 = blk * BLK + ft * 128
            e_ps = psE.tile([128, ncl], F32, tag="ep")
            for h0 in range(0, ncl, 512):
                hw = min(512, ncl - h0)
                nc.tensor.matmul(e_ps[:, h0:h0 + hw], augf[:, f0:f0 + 128],
                                 augc[:, h0:h0 + hw],
                                 start=True, stop=True)
            # r_un = 1/(d2 + 1e-12) straight from PSUM (e = -d2; the -1e-12
            # bias is the reference's +eps guard). Reciprocal is monotone-
            # decreasing in d2, so nearest-3 = largest-3 of r_un.
            r_un = pe_sb.tile([128, ncl], F32, tag="r_un")
            if cfg.get("t_norsc", False):
                nc.scalar.activation(r_un[:], e_ps[:], AF.Copy, bias=-1e-12,
                                     scale=-1.0)
            else:
                act_recip(r_un[:], e_ps[:], -1.0, bias=-1e-12)
            m8 = psml.tile([128, 8], F32, tag="m8")
            if cfg.get("t_nomax", False):
                nc.vector.tensor_reduce(m8[:, 0:1], r_un[:], AX.X, OP.max)
                nc.vector.tensor_copy(m8[:, 1:2], m8[:, 0:1])
                nc.vector.tensor_copy(m8[:, 2:3], m8[:, 0:1])
            else:
                nc.vector.max(m8[:], r_un[:])
            sw = psml.tile([128, 1], F32, tag="sw")
            nc.vector.tensor_reduce(sw[:], m8[:, 0:3], AX.X, OP.add)
            invs = psml.tile([128, 1], F32, tag="invs")
            nc.vector.reciprocal(invs[:], sw[:])
            wun = prsc.tile([128, ncl], F32, tag="wun")
            nc.vector.scalar_tensor_tensor(wun[:], r_un[:], m8[:, 2:3],
                                           r_un[:], OP.is_ge, OP.mult)
            w16 = pw16.tile([128, ncl], F16, tag="w16")
            norm_eng = nc.vector if cfg.get("t_dvenorm", False) else nc.gpsimd
            norm_eng.tensor_scalar(w16[:], wun[:], invs[:, 0:1], None, OP.mult)
            if cfg.get("t_notrans", False):
                nc.sync.dma_start(wt[:, :, ft * 128:(ft + 1) * 128], w16[:])
            else:
                nc.sync.dma_start(wt[:, :, ft * 128:(ft + 1) * 128],
                                  w16[:], transpose=True)
            if dbg:
                nc.sync.dma_start(EDBG[cl, f0:f0 + 128, :], r_un[:])
                nc.sync.dma_start(M8DBG[cl, blk, :, ft * 8:(ft + 1) * 8], m8[:])
        return wt

    def emit_mlp(cl, blk, res, wt):
        augf, augc, fsb = res
        pim = psI.tile([128, 2, BLK], F32, tag="pim")
        for m in range(2):
            for cc in range(ncc):
                nc.tensor.matmul(pim[:, m, :], fsb[:, cc, m * 128:(m + 1) * 128],
                                 wt[:, cc, :], start=(cc == 0), stop=(cc == ncc - 1))
        ht01 = pht.tile([128, 2, BLK], F16, tag="ht01")
        nc.scalar.copy(ht01[:], pim[:])
        hts = [ht01[:, 0, :], ht01[:, 1, :]]
        ht23 = pht.tile([128, 2, BLK], F16, tag="ht23")
        nc.sync.dma_start(ht23[:], XSKT[cl, :, blk * BLK:(blk + 1) * BLK]
                          .rearrange("(a p) f -> p a f", p=128))
        hts += [ht23[:, 0, :], ht23[:, 1, :]]

        h1s = []
        for j in range(4):
            p1 = psMM.tile([128, BLK], F32, tag="pmm")
            for k in range(4):
                nc.tensor.matmul(p1[:], w1sb[:, k, j * 128:(j + 1) * 128],
                                 hts[k], start=(k == 0), stop=(k == 3))
            h1 = ph1.tile([128, BLK], F16, tag=f"h1{j}")
            nc.scalar.activation(h1[:], p1[:], AF.Relu,
                                 bias=b1sb[:, j:j + 1], scale=1.0)
            h1s.append(h1)

        for m in range(2):
            p2 = psMM.tile([128, BLK], F32, tag="pmm")
            for j in range(4):
                nc.tensor.matmul(p2[:], w2sb[:, j, m * 128:(m + 1) * 128],
                                 h1s[j][:], start=(j == 0), stop=(j == 3))
            h2 = ph2.tile([128, BLK], F32, tag="h2")
            nc.scalar.activation(h2[:], p2[:], AF.Relu,
                                 bias=b2sb[:, m:m + 1], scale=1.0)
            nc.sync.dma_start(OUTD[cl, m * 128:(m + 1) * 128,
                                   blk * BLK:(blk + 1) * BLK], h2[:])

    # 1-block-deep software pipeline: selection chain for item i runs while
    # the PE consumes item i-1's weights in interp/MLP.
    DEPTH = 2
    items = [(cl, blk) for cl in range(bpc) for blk in range(nblk)]
    items = items * cfg.get("repeat", 1)
    state = {}
    res = None
    for i in range(len(items) + DEPTH):
        if i < len(items):
            cl, blk = items[i]
            if i == 0 or (blk == 0 and items[i - 1][0] != cl):
                res = load_cloud(cl)
            state[i] = (items[i], res, emit_chain(cl, blk, res))
        if i >= DEPTH:
            (pcl_, pblk), pres, pwt_t = state.pop(i - DEPTH)
            emit_mlp(pcl_, pblk, pres, pwt_t)

    ctx.close()


def host_prep(inputs, cfg):
    """Build per-core input maps from full inputs."""
    ncl, nfl, bpc, ncores = cfg["NC"], cfg["NF"], cfg["BPC"], cfg["NCORES"]
    nb = bpc * ncores
    x = np.asarray(inputs["x"], dtype=np.float32).reshape(nb, ncl, C)
    pos = np.asarray(inputs["pos"], dtype=np.float32).reshape(nb, ncl, 3)
    x_skip = np.asarray(inputs["x_skip"], dtype=np.float32).reshape(nb, nfl, C)
    pos_skip = np.asarray(inputs["pos_skip"], dtype=np.float32).reshape(nb, nfl, 3)
    W1 = np.asarray(inputs["W1"], dtype=np.float32)
    W2 = np.asarray(inputs["W2"], dtype=np.float32)
    b1 = np.asarray(inputs["b1"], dtype=np.float32)
    b2 = np.asarray(inputs["b2"], dtype=np.float32)

    maps = []
    half = np.float32(0.5)
    W1h = W1.astype(np.float16)
    W2h = W2.astype(np.float16)
    b1T = np.ascontiguousarray(b1.reshape(4, 128).T)
    b2T = np.ascontiguousarray(b2.reshape(2, 128).T)
    for core in range(ncores):
        af = np.zeros((bpc, KAUG, nfl), dtype=bfl)
        ac = np.zeros((bpc, KAUG, ncl), dtype=bfl)
        ft = np.zeros((bpc, ncl, C), dtype=np.float16)
        xt = np.zeros((bpc, C, nfl), dtype=np.float16)
        for i in range(bpc):
            b = core * bpc + i
            af[i], ac[i] = _build_aug(pos_skip[b] - half, pos[b] - half)
            ft[i] = x[b].astype(np.float16)
            xt[i] = x_skip[b].T.astype(np.float16)
        maps.append({
            "AUGF": af, "AUGC": ac, "FT": ft, "XSKT": xt,
            "W1D": W1h, "W2D": W2h, "B1D": b1T, "B2D": b2T,
        })
    return maps


_compiled = {}


def _get_compiled(cfg_key, cfg):
    if cfg_key in _compiled:
        return _compiled[cfg_key]
    import concourse.tile as tile
    from concourse import bacc
    nc = bacc.Bacc("TRN2", target_bir_lowering=False, debug=False,
                   num_devices=cfg["NCORES"])
    with tile.TileContext(nc) as tc:
        build_core_kernel(nc, tc, cfg)
    nc.compile()
    _compiled[cfg_key] = nc
    return nc


def kernel(**inputs):
    cfg = {"NC": NC, "NF": NF, "BPC": BPC, "NCORES": NCORES}
    from concourse.bass_utils import run_bass_kernel_spmd
    nc = _get_compiled("full", cfg)
    maps = host_prep(inputs, cfg)
    res = run_bass_kernel_spmd(nc, maps, list(range(NCORES)))
    h = np.zeros((B * NF, C), dtype=np.float32)
    for core in range(NCORES):
        out = res.results[core]["OUTD"]        # [BPC, C, NF]
        for i in range(BPC):
            b = core * BPC + i
            h[b * NF:(b + 1) * NF] = out[i].T
    pos_skip = np.asarray(inputs["pos_skip"])
    batch_skip = np.asarray(inputs["batch_skip"])
    return (h, pos_skip, batch_skip)


# revision 21
# speedup vs baseline: 3.4076x; 3.4076x over previous
"""Trainium2 Bass kernel for nn_FPModule (retrieval_knn feature propagation).

Pipeline per core (2 point clouds each, data-parallel over 8 cores):
  1. e = -squared-distance matrix via PE matmul with 3-way bf16-split
     augmented coordinates (exact products, fp32 psum accumulation).
  2. Top-3 nearest coarse points per fine point via DVE InstMax (top-8 sorted).
  3. Normalized inverse-distance weights built as a sparse-row matrix W (fp16)
     with a fused compare+multiply; W transposed on the DMA xbar.
  4. interp^T = F^T-chunks @ W^T on PE (fp16), concat with x_skip^T (host-prep),
     then the 2-layer MLP entirely in transposed activation layout.
Output assembled/transposed on host. All dtype-critical selection math is fp32.
"""
import sys, os
for _p in ("/opt/trn_rl_repo", "/root/.axon_site/_ro/trn_rl_repo"):
    if os.path.isdir(_p) and _p not in sys.path:
        sys.path.insert(0, _p)

import numpy as np
import ml_dtypes

B, NC, NF, C = 16, 1024, 4096, 256
NCORES = 8
BPC = B // NCORES          # clouds per core
KAUG = 36                  # augmented contraction rows
BLK = 512                  # fine-block size
EPS = np.float32(1e-12)

bfl = ml_dtypes.bfloat16


def _split3(v64):
    """3-way bf16 split of fp64 values (captures ~24 mantissa bits)."""
    a = v64.astype(bfl)
    r = v64 - a.astype(np.float64)
    b = r.astype(bfl)
    r2 = r - b.astype(np.float64)
    c = r2.astype(bfl)
    return a, b, c


def _build_aug(pf32, pc32):
    """aug_fT [KAUG, NFl] bf16, aug_cT [KAUG, NCl] bf16 for e = -|pf-pc|^2.

    Per coordinate d, 12 k-rows in psum-accumulation order:
      6 cross pairs of splits of (2*pf_d) x splits of (pc_d):
         (fa,ca),(fa,cb),(fb,ca),(fa,cc),(fc,ca),(fb,cb)
      3 rows: splits of (-pf_d^2) x 1
      3 rows: 1 x splits of (-pc_d^2)
    """
    NFl, NCl = pf32.shape[0], pc32.shape[0]
    af = np.zeros((KAUG, NFl), dtype=bfl)
    ac = np.zeros((KAUG, NCl), dtype=bfl)
    onesf = np.ones(NFl, dtype=bfl)
    onesc = np.ones(NCl, dtype=bfl)
    pf = pf32.astype(np.float64)
    pc = pc32.astype(np.float64)
    k = 0
    for d in range(3):
        fa, fb, fc = _split3(2.0 * pf[:, d])
        ca, cb, cc_ = _split3(pc[:, d])
        for (u, v) in ((fa, ca), (fa, cb), (fb, ca), (fa, cc_), (fc, ca), (fb, cb)):
            af[k] = u; ac[k] = v; k += 1
        s1, s2, s3 = _split3(-(pf[:, d] ** 2))
        for s in (s1, s2, s3):
            af[k] = s; ac[k] = onesc; k += 1
        t1, t2, t3 = _split3(-(pc[:, d] ** 2))
        for t in (t1, t2, t3):
            af[k] = onesf; ac[k] = t; k += 1
    assert k == KAUG
    return af, ac


def build_core_kernel(nc, tc, cfg):
    """Emit one core's kernel (BPC clouds) inside an open TileContext."""
    import concourse.bass as bass
    from concourse import mybir

    F32 = mybir.dt.float32
    F16 = mybir.dt.float16
    BF16 = mybir.dt.bfloat16
    AF = mybir.ActivationFunctionType
    OP = mybir.AluOpType
    AX = mybir.AxisListType

    ncl, nfl, bpc = cfg["NC"], cfg["NF"], cfg["BPC"]
    nblk = nfl // BLK
    ncc = ncl // 128          # coarse chunks
    dbg = cfg.get("debug_outs", False)

    def act_recip(out, in_, scale, bias=0.0):
        """Raw ACT Reciprocal out = 1/(in*scale + bias); table accuracy ~1e-5
        (measured), far below the fp16 quantization of the consumers."""
        eng = nc.scalar
        inputs = [eng.lower_ap(in_)]
        for arg in (bias, scale, 0.0):   # bias, scale, alpha
            if isinstance(arg, bass.AP):
                inputs.append(eng.lower_ap(arg))
            else:
                inputs.append(mybir.ImmediateValue(dtype=mybir.dt.float32, value=arg))
        return eng.add_instruction(
            mybir.InstActivation(
                name=eng.bass.get_next_instruction_name(),
                func=AF.Reciprocal,
                ins=inputs,
                outs=[eng.lower_ap(out)],
            )
        )

    AUGF = nc.dram_tensor("AUGF", [bpc, KAUG, nfl], BF16, kind="ExternalInput").ap()
    AUGC = nc.dram_tensor("AUGC", [bpc, KAUG, ncl], BF16, kind="ExternalInput").ap()
    FT = nc.dram_tensor("FT", [bpc, ncl, C], F16, kind="ExternalInput").ap()
    XSKT = nc.dram_tensor("XSKT", [bpc, C, nfl], F16, kind="ExternalInput").ap()
    W1D = nc.dram_tensor("W1D", [512, 512], F16, kind="ExternalInput").ap()
    W2D = nc.dram_tensor("W2D", [512, 256], F16, kind="ExternalInput").ap()
    B1D = nc.dram_tensor("B1D", [128, 4], F32, kind="ExternalInput").ap()
    B2D = nc.dram_tensor("B2D", [128, 2], F32, kind="ExternalInput").ap()
    OUTD = nc.dram_tensor("OUTD", [bpc, C, nfl], F32, kind="ExternalOutput").ap()
    if dbg:
        EDBG = nc.dram_tensor("EDBG", [bpc, nfl, ncl], F32, kind="ExternalOutput").ap()
        M8DBG = nc.dram_tensor("M8DBG", [bpc, nblk, 128, 32], F32, kind="ExternalOutput").ap()
        WTDBG = nc.dram_tensor("WTDBG", [bpc, nblk, 128, ncc, BLK], F16, kind="ExternalOutput").ap()

    use_div = cfg.get("use_div", False)

    import contextlib
    ctx = contextlib.ExitStack()
    cst = ctx.enter_context(tc.tile_pool(name="cst", bufs=1))
    pcl = ctx.enter_context(tc.tile_pool(name="pcl", bufs=2))
    pe_sb = ctx.enter_context(tc.tile_pool(name="pe_sb", bufs=8))
    pw16 = ctx.enter_context(tc.tile_pool(name="pw16", bufs=4))
    pwt = ctx.enter_context(tc.tile_pool(name="pwt", bufs=4))
    psml = ctx.enter_context(tc.tile_pool(name="psml", bufs=10))
    prsc = ctx.enter_context(tc.tile_pool(name="prsc", bufs=4))
    pht = ctx.enter_context(tc.tile_pool(name="pht", bufs=3))
    ph1 = ctx.enter_context(tc.tile_pool(name="ph1", bufs=2))
    ph2 = ctx.enter_context(tc.tile_pool(name="ph2", bufs=2))
    psE = ctx.enter_context(tc.tile_pool(name="psE", bufs=2, space="PSUM"))
    psI = ctx.enter_context(tc.tile_pool(name="psI", bufs=1, space="PSUM"))
    psMM = ctx.enter_context(tc.tile_pool(name="psMM", bufs=2, space="PSUM"))

    # constants
    w1sb = cst.tile([128, 4, 512], F16, tag="w1sb")
    for kc in range(4):
        nc.sync.dma_start(w1sb[:, kc, :], W1D[kc * 128:(kc + 1) * 128, :])
    w2sb = cst.tile([128, 4, 256], F16, tag="w2sb")
    for kc in range(4):
        nc.sync.dma_start(w2sb[:, kc, :], W2D[kc * 128:(kc + 1) * 128, :])
    b1sb = cst.tile([128, 4], F32, tag="b1sb")
    nc.sync.dma_start(b1sb[:], B1D[:])
    b2sb = cst.tile([128, 2], F32, tag="b2sb")
    nc.sync.dma_start(b2sb[:], B2D[:])

    def load_cloud(cl):
        augf = pcl.tile([KAUG, nfl], BF16, tag="augf")
        nc.sync.dma_start(augf[:], AUGF[cl])
        augc = pcl.tile([KAUG, ncl], BF16, tag="augc")
        nc.sync.dma_start(augc[:], AUGC[cl])
        fsb = pcl.tile([128, ncc, C], F16, tag="fsb")
        for cc in range(ncc):
            nc.sync.dma_start(fsb[:, cc, :], FT[cl, cc * 128:(cc + 1) * 128, :])
        return augf, augc, fsb

    def emit_chain(cl, blk, res):
        augf, augc, fsb = res
        wt = pwt.tile([128, ncc, BLK], F16, tag="wt")
        for ft in range(4):
            f0 = blk * BLK + ft * 128
            e_ps = psE.tile([128, ncl], F32, tag="ep")
            for h0 in range(0, ncl, 512):
                hw = min(512, ncl - h0)
                nc.tensor.matmul(e_ps[:, h0:h0 + hw], augf[:, f0:f0 + 128],
                                 augc[:, h0:h0 + hw],
                                 start=True, stop=True)
            # r_un = 1/(d2 + 1e-12) straight from PSUM (e = -d2; the -1e-12
            # bias is the reference's +eps guard). Reciprocal is monotone-
            # decreasing in d2, so nearest-3 = largest-3 of r_un.
            r_un = pe_sb.tile([128, ncl], F32, tag="r_un")
            if cfg.get("t_norsc", False):
                nc.scalar.activation(r_un[:], e_ps[:], AF.Copy, bias=-1e-12,
                                     scale=-1.0)
            else:
                act_recip(r_un[:], e_ps[:], -1.0, bias=-1e-12)
            m8 = psml.tile([128, 8], F32, tag="m8")
            if cfg.get("t_nomax", False):
                nc.vector.tensor_reduce(m8[:, 0:1], r_un[:], AX.X, OP.max)
                nc.vector.tensor_copy(m8[:, 1:2], m8[:, 0:1])
                nc.vector.tensor_copy(m8[:, 2:3], m8[:, 0:1])
            else:
                nc.vector.max(m8[:], r_un[:])
            sw = psml.tile([128, 1], F32, tag="sw")
            nc.vector.tensor_reduce(sw[:], m8[:, 0:3], AX.X, OP.add)
            invs = psml.tile([128, 1], F32, tag="invs")
            nc.vector.reciprocal(invs[:], sw[:])
            wun = prsc.tile([128, ncl], F32, tag="wun")
            nc.vector.scalar_tensor_tensor(wun[:], r_un[:], m8[:, 2:3],
                                           r_un[:], OP.is_ge, OP.mult)
            w16 = pw16.tile([128, ncl], F16, tag="w16")
            if ft == 0:
                # ACT does 1 of 4 norm passes (Copy with per-partition scale)
                nc.scalar.activation(w16[:], wun[:], AF.Copy, bias=0.0,
                                     scale=invs[:, 0:1])
            else:
                nc.vector.tensor_scalar(w16[:], wun[:], invs[:, 0:1], None,
                                        OP.mult)
            if cfg.get("t_notrans", False):
                nc.sync.dma_start(wt[:, :, ft * 128:(ft + 1) * 128], w16[:])
            else:
                nc.sync.dma_start(wt[:, :, ft * 128:(ft + 1) * 128],
                                  w16[:], transpose=True)
            if dbg:
                nc.sync.dma_start(EDBG[cl, f0:f0 + 128, :], r_un[:])
                nc.sync.dma_start(M8DBG[cl, blk, :, ft * 8:(ft + 1) * 8], m8[:])
        return wt

    def emit_mlp(cl, blk, res, wt):
        augf, augc, fsb = res
        pim = psI.tile([128, 2, BLK], F32, tag="pim")
        for m in range(2):
            for cc in range(ncc):
                nc.tensor.matmul(pim[:, m, :], fsb[:, cc, m * 128:(m + 1) * 128],
                                 wt[:, cc, :], start=(cc == 0), stop=(cc == ncc - 1))
        ht01 = pht.tile([128, 2, BLK], F16, tag="ht01")
        nc.scalar.copy(ht01[:], pim[:])
        hts = [ht01[:, 0, :], ht01[:, 1, :]]
        ht23 = pht.tile([128, 2, BLK], F16, tag="ht23")
        nc.sync.dma_start(ht23[:], XSKT[cl, :, blk * BLK:(blk + 1) * BLK]
                          .rearrange("(a p) f -> p a f", p=128))
        hts += [ht23[:, 0, :], ht23[:, 1, :]]

        h1s = []
        for j in range(4):
            p1 = psMM.tile([128, BLK], F32, tag="pmm")
            for k in range(4):
                nc.tensor.matmul(p1[:], w1sb[:, k, j * 128:(j + 1) * 128],
                                 hts[k], start=(k == 0), stop=(k == 3))
            h1 = ph1.tile([128, BLK], F16, tag=f"h1{j}")
            nc.scalar.activation(h1[:], p1[:], AF.Relu,
                                 bias=b1sb[:, j:j + 1], scale=1.0)
            h1s.append(h1)

        for m in range(2):
            p2 = psMM.tile([128, BLK], F32, tag="pmm")
            for j in range(4):
                nc.tensor.matmul(p2[:], w2sb[:, j, m * 128:(m + 1) * 128],
                                 h1s[j][:], start=(j == 0), stop=(j == 3))
            h2 = ph2.tile([128, BLK], F32, tag="h2")
            nc.scalar.activation(h2[:], p2[:], AF.Relu,
                                 bias=b2sb[:, m:m + 1], scale=1.0)
            nc.sync.dma_start(OUTD[cl, m * 128:(m + 1) * 128,
                                   blk * BLK:(blk + 1) * BLK], h2[:])

    # 1-block-deep software pipeline: selection chain for item i runs while
    # the PE consumes item i-1's weights in interp/MLP.
    DEPTH = 2
    items = [(cl, blk) for cl in range(bpc) for blk in range(nblk)]
    items = items * cfg.get("repeat", 1)
    state = {}
    res = None
    for i in range(len(items) + DEPTH):
        if i < len(items):
            cl, blk = items[i]
            if i == 0 or (blk == 0 and items[i - 1][0] != cl):
                res = load_cloud(cl)
            state[i] = (items[i], res, emit_chain(cl, blk, res))
        if i >= DEPTH:
            (pcl_, pblk), pres, pwt_t = state.pop(i - DEPTH)
            emit_mlp(pcl_, pblk, pres, pwt_t)

    ctx.close()


def host_prep(inputs, cfg):
    """Build per-core input maps from full inputs."""
    ncl, nfl, bpc, ncores = cfg["NC"], cfg["NF"], cfg["BPC"], cfg["NCORES"]
    nb = bpc * ncores
    x = np.asarray(inputs["x"], dtype=np.float32).reshape(nb, ncl, C)
    pos = np.asarray(inputs["pos"], dtype=np.float32).reshape(nb, ncl, 3)
    x_skip = np.asarray(inputs["x_skip"], dtype=np.float32).reshape(nb, nfl, C)
    pos_skip = np.asarray(inputs["pos_skip"], dtype=np.float32).reshape(nb, nfl, 3)
    W1 = np.asarray(inputs["W1"], dtype=np.float32)
    W2 = np.asarray(inputs["W2"], dtype=np.float32)
    b1 = np.asarray(inputs["b1"], dtype=np.float32)
    b2 = np.asarray(inputs["b2"], dtype=np.float32)

    maps = []
    half = np.float32(0.5)
    W1h = W1.astype(np.float16)
    W2h = W2.astype(np.float16)
    b1T = np.ascontiguousarray(b1.reshape(4, 128).T)
    b2T = np.ascontiguousarray(b2.reshape(2, 128).T)
    for core in range(ncores):
        af = np.zeros((bpc, KAUG, nfl), dtype=bfl)
        ac = np.zeros((bpc, KAUG, ncl), dtype=bfl)
        ft = np.zeros((bpc, ncl, C), dtype=np.float16)
        xt = np.zeros((bpc, C, nfl), dtype=np.float16)
        for i in range(bpc):
            b = core * bpc + i
            af[i], ac[i] = _build_aug(pos_skip[b] - half, pos[b] - half)
            ft[i] = x[b].astype(np.float16)
            xt[i] = x_skip[b].T.astype(np.float16)
        maps.append({
            "AUGF": af, "AUGC": ac, "FT": ft, "XSKT": xt,
            "W1D": W1h, "W2D": W2h, "B1D": b1T, "B2D": b2T,
        })
    return maps


_compiled = {}


def _get_compiled(cfg_key, cfg):
    if cfg_key in _compiled:
        return _compiled[cfg_key]
    import concourse.tile as tile
    from concourse import bacc
    nc = bacc.Bacc("TRN2", target_bir_lowering=False, debug=False,
                   num_devices=cfg["NCORES"])
    with tile.TileContext(nc) as tc:
        build_core_kernel(nc, tc, cfg)
    nc.compile()
    _compiled[cfg_key] = nc
    return nc


def kernel(**inputs):
    cfg = {"NC": NC, "NF": NF, "BPC": BPC, "NCORES": NCORES}
    from concourse.bass_utils import run_bass_kernel_spmd
    nc = _get_compiled("full", cfg)
    maps = host_prep(inputs, cfg)
    res = run_bass_kernel_spmd(nc, maps, list(range(NCORES)))
    h = np.zeros((B * NF, C), dtype=np.float32)
    for core in range(NCORES):
        out = res.results[core]["OUTD"]        # [BPC, C, NF]
        for i in range(BPC):
            b = core * BPC + i
            h[b * NF:(b + 1) * NF] = out[i].T
    pos_skip = np.asarray(inputs["pos_skip"])
    batch_skip = np.asarray(inputs["batch_skip"])
    return (h, pos_skip, batch_skip)
